# revision 6
# baseline (speedup 1.0000x reference)
"""2-layer GraphSAGE (mean aggregation) over 8 TRN2 NeuronCores — v4.

v4 vs v3:
- num_swdge_queues=4 with round-robin queue_num on the L2 dma_gather
  calls: each SWDGE queue has a dedicated pair of Q7 DSP cores, so up
  to 4 gathers generate descriptors concurrently (the v3 gather stream
  was fully serial on one core pair at ~7.7ns/idx and was 86% of the
  kernel span).
- L2 gathers single rows (256B) instead of pairs (512B): pairing only
  deduped ~2% of indices on this random graph while doubling the
  matmul count, the DVE one-hot columns, and the gather bytes. int16
  index range is handled by making indices relative to a per-pass base
  row (each pass's gather source window spans <= 32768 h_full rows).
- One-hot IS_EQ builds are batched 16 slabs per DVE instruction.
- PSUM->SBUF copies (output tile, hT transpose, spill init) moved to
  the otherwise-idle Scalar (Activation) engine.

Layer 1 is host-pregathered (the gather pattern depends only on
edge_index): the device streams pre-packed slab tables with plain HWDGE
DMA. Aggregation matmuls accumulate agg^T[feat, dst] with the gathered
slab as lhsT and the one-hot as rhs; mean scaling uses a
host-replicated [P, npad] inv-degree table.
"""

import sys

for _p in ("/opt/trn_rl_repo",):
    if _p not in sys.path:
        sys.path.insert(0, _p)

import hashlib
import numpy as np
import ml_dtypes

BF16 = ml_dtypes.bfloat16
P = 128
D = 128

SLAB_BUDGET = 20   # max 128-slot slabs per L2 gather call
NQUEUES = 4
IDX_SPAN = 32768   # int16 index reach per gather call (rows)


# --------------------------------------------------------------------------
# Host-side schedule construction
# --------------------------------------------------------------------------

def build_meta(src, dst, N, ncores):
    src = np.asarray(src, np.int64)
    dst = np.asarray(dst, np.int64)
    npc = N // ncores
    assert npc * ncores == N, (N, ncores)
    blocks = -(-npc // P)
    npad = blocks * P
    assert npc < npad, "need pad ranks for layer-2 zero sentinels"

    # uneven source chunks (in blocks): small first chunks so the first
    # AllGather — which gates the start of dma_gather descriptor
    # generation — completes as early as possible. Consecutive chunk
    # pairs stay under IDX_SPAN rows so per-pass relative int16 indices
    # can reach back across one chunk boundary.
    cuts = sorted({max(1, (blocks * num) // den) for num, den in
                   ((5, 49), (14, 49), (27, 49), (41, 49), (1, 1))})
    if cuts[-1] != blocks:
        cuts.append(blocks)
    cum_blocks = [0] + cuts
    nchunk = len(cuts)
    chunk_ranks = [cum_blocks[i + 1] * P - cum_blocks[i] * P
                   for i in range(nchunk)]
    cum_ranks = [cb * P for cb in cum_blocks]            # per-core rank cuts
    row_off = [ncores * cr for cr in cum_ranks]          # h_full row offsets
    assert npc > cum_ranks[-2], "pad ranks must fall in the last chunk"
    # per-pass gather base rows (indices are relative to these)
    base_rows = [max(0, row_off[p + 1] - IDX_SPAN) for p in range(nchunk)]
    for p in range(nchunk):
        assert row_off[p + 1] - base_rows[p] <= IDX_SPAN

    deg = np.bincount(dst, minlength=N)
    invcnt = (1.0 / np.maximum(deg, 1.0)).astype(np.float32)

    perm = -np.ones((ncores, npad), np.int64)
    for c in range(ncores):
        perm[c, :npc] = np.arange(c * npc, (c + 1) * npc)
    rank = np.mod(np.arange(N), npc)
    core_of = np.arange(N) // npc
    q_of = np.searchsorted(cum_ranks, rank, side="right") - 1
    q_of = np.minimum(q_of, nchunk - 1)
    cr = np.asarray(cum_ranks[:-1], np.int64)[q_of]
    crk = np.asarray(chunk_ranks, np.int64)[q_of]
    pos = (np.asarray(row_off[:-1], np.int64)[q_of]
           + core_of * crk + (rank - cr))  # chunk-major h_full row

    # ---------------- Layer 1: host-pregathered, unpaired slots ----------
    per_core_l1 = []
    nch1 = np.zeros(blocks, np.int64)
    for c in range(ncores):
        sel = core_of[dst] == c
        s_c = src[sel]
        r = rank[dst[sel]]
        w = r // P
        dl = r % P
        order = np.argsort(w, kind="stable")
        sw = w[order]
        cntw = np.bincount(sw, minlength=blocks)
        np.maximum(nch1, -(-cntw // P), out=nch1)
        per_core_l1.append((s_c[order], sw, dl[order]))
    nch1 = np.maximum(nch1, 1)
    slab_of1 = np.concatenate([[0], np.cumsum(nch1)[:-1]])
    S1 = int(nch1.sum())

    src_slots = []
    dstl1 = []
    for c in range(ncores):
        s_sorted, sw, dl_sorted = per_core_l1[c]
        flat_src = -np.ones(S1 * P, np.int64)
        flat_dl = np.full(S1 * P, 255, np.int64)
        wstart = np.searchsorted(sw, np.arange(blocks))
        posn = slab_of1[sw] * P + (np.arange(len(sw)) - wstart[sw])
        flat_src[posn] = s_sorted
        flat_dl[posn] = dl_sorted
        src_slots.append(flat_src)
        dstl1.append(np.ascontiguousarray(
            flat_dl.reshape(S1, P).T.astype(BF16)))

    l1_windows = [{"w": b, "slabs": (int(slab_of1[b]), int(slab_of1[b] + nch1[b]))}
                  for b in range(blocks)]

    # ---------------- Layer 2: single-row slots, passes by source chunk --
    NPASS_L2 = nchunk
    nch = np.zeros(blocks, np.int64)
    per_core = []
    for c in range(ncores):
        sel = core_of[dst] == c
        s_c = src[sel]
        r = rank[dst[sel]]
        row = pos[s_c]                      # h_full source row per edge
        w = r // P
        dl = r % P
        okey = (w << 16) | row
        order = np.argsort(okey, kind="stable")
        sk = okey[order]
        # one slot per edge, sorted by (window, row)
        uw = (sk >> 16).astype(np.int64)
        cntw = np.bincount(uw, minlength=blocks)
        np.maximum(nch, -(-cntw // P), out=nch)
        per_core.append((sk & 0xFFFF, uw, dl[order]))
    nch = np.maximum(nch, 1)

    # shared per-(window, slab) pass assignment: max over cores of the
    # chunk of the slab's last real slot for that core
    slab_pass = [np.zeros(int(nch[b]), np.int64) for b in range(blocks)]
    for c in range(ncores):
        rows, uw, dl = per_core[c]
        uq = np.searchsorted(row_off, rows, side="right") - 1
        wfirst = np.searchsorted(uw, np.arange(blocks))
        wcnt = np.bincount(uw, minlength=blocks)
        for b in range(blocks):
            n = int(wcnt[b])
            if n == 0:
                continue
            qs = uq[wfirst[b]:wfirst[b] + n]
            for j in range(int(nch[b])):
                if 128 * j >= n:
                    break
                last = min(128 * (j + 1) - 1, n - 1)
                slab_pass[b][j] = max(slab_pass[b][j], qs[last])

    # global slab order: pass-major, then window
    slab_gid = [np.zeros(int(nch[b]), np.int64) for b in range(blocks)]
    slab_pass_flat = {}
    runs_by_pass = [[] for _ in range(NPASS_L2)]  # (w, j0, j1, t0)
    t = 0
    for p in range(NPASS_L2):
        for b in range(blocks):
            js = np.nonzero(slab_pass[b] == p)[0]
            if len(js) == 0:
                continue
            j0, j1 = int(js[0]), int(js[-1]) + 1
            assert list(js) == list(range(j0, j1))
            slab_gid[b][j0:j1] = np.arange(t, t + (j1 - j0))
            for g in range(t, t + (j1 - j0)):
                slab_pass_flat[g] = p
            runs_by_pass[p].append((b, j0, j1, t))
            t += j1 - j0
    total_slabs = t
    assert total_slabs == int(nch.sum())

    # groups (gather calls): pack whole runs under SLAB_BUDGET, per pass
    groups_by_pass = []
    for p in range(NPASS_L2):
        groups = []
        cur = None
        for (b, j0, j1, t0) in runs_by_pass[p]:
            n = j1 - j0
            assert n <= SLAB_BUDGET
            if cur is None or cur["nslab"] + n > SLAB_BUDGET:
                cur = {"base": t0, "nslab": 0, "windows": []}
                groups.append(cur)
            cur["windows"].append(
                {"w": b, "chunks": [(t0 - cur["base"] + j, t0 + j)
                                    for j in range(n)]})
            cur["nslab"] += n
        for g in groups:
            g["col"] = g["base"] * 8
            g["NI"] = g["nslab"] * P
        groups_by_pass.append(groups)

    # per-window pass bookkeeping
    wpasses = [[] for _ in range(blocks)]
    for p in range(NPASS_L2):
        for (b, j0, j1, t0) in runs_by_pass[p]:
            wpasses[b].append(p)

    # per-core idx / dstl tables in the global slab order; indices are
    # relative to base_rows[pass(slab)]
    idx2 = []
    dstl2 = []
    for c in range(ncores):
        rows, uw, dl = per_core[c]
        flat = np.zeros(total_slabs * P, np.int64)
        dst2 = np.full(total_slabs * P, 255, np.int64)
        wfirst = np.searchsorted(uw, np.arange(blocks))
        iw = np.arange(len(uw)) - wfirst[uw]   # position within window
        jloc = iw // P
        within = iw % P
        gsl = np.empty(len(uw), np.int64)
        for b in range(blocks):
            m = uw == b
            if m.any():
                gsl[m] = slab_gid[b][jloc[m]]
        spos = gsl * P + within
        gbase = np.asarray([base_rows[slab_pass_flat[g]]
                            for g in range(total_slabs)], np.int64)
        rel = rows - gbase[gsl]
        assert (rel >= 0).all() and (rel < IDX_SPAN).all(), \
            "slot row outside its pass's index window"
        flat[spos] = rel
        dst2[spos] = dl
        idx2.append(np.ascontiguousarray(
            np.tile(flat.reshape(-1, 16).T.astype(np.int16), (8, 1))))
        dstl2.append(np.ascontiguousarray(
            dst2.reshape(total_slabs, P).T.astype(BF16)))

    layer2 = {"groups_by_pass": groups_by_pass, "C": total_slabs * 8,
              "TC": total_slabs, "wpasses": wpasses}

    return {
        "N": N, "ncores": ncores, "blocks": blocks, "npad": npad,
        "cum_ranks": cum_ranks, "row_off": row_off, "npass": NPASS_L2,
        "base_rows": base_rows,
        "perm": perm, "invcnt": invcnt,
        "S1": S1, "l1_windows": l1_windows,
        "src_slots": src_slots, "dstl1": dstl1,
        "layer2": layer2, "idx2": idx2, "dstl2": dstl2,
    }


# --------------------------------------------------------------------------
# Bass graph
# --------------------------------------------------------------------------

def build_nc(meta):
    from concourse import bacc, mybir
    from concourse.tile import TileContext

    dt = mybir.dt
    ALU = mybir.AluOpType
    ACT = mybir.ActivationFunctionType
    m = meta
    npad, ncores, blocks = m["npad"], m["ncores"], m["blocks"]
    cum_ranks, row_off = m["cum_ranks"], m["row_off"]
    base_rows = m["base_rows"]
    npass = m["npass"]
    S1 = m["S1"]
    l2 = m["layer2"]

    nc = bacc.Bacc(num_swdge_queues=NQUEUES)

    gt1 = nc.declare_dram_parameter("gt1", [P, S1 * D], dt.bfloat16, isOutput=False)
    dstl1 = nc.declare_dram_parameter("dstl1", [P, S1], dt.bfloat16, isOutput=False)
    idx2 = nc.declare_dram_parameter("idx2", [P, l2["C"]], dt.int16, isOutput=False)
    dstl2 = nc.declare_dram_parameter("dstl2", [P, l2["TC"]], dt.bfloat16, isOutput=False)
    xT = nc.declare_dram_parameter("xT", [P, npad], dt.bfloat16, isOutput=False)
    m2r = nc.declare_dram_parameter("m2r", [npad, D], dt.bfloat16, isOutput=False)
    invcb = nc.declare_dram_parameter("invcb", [P, npad], dt.float32, isOutput=False)
    iota32 = nc.declare_dram_parameter("iota32", [P, 32 * P], dt.bfloat16, isOutput=False)
    onesr = nc.declare_dram_parameter("onesr", [1, P], dt.bfloat16, isOutput=False)
    ident = nc.declare_dram_parameter("ident", [P, P], dt.bfloat16, isOutput=False)
    w1l = nc.declare_dram_parameter("w1l", [P, P], dt.bfloat16, isOutput=False)
    w1r = nc.declare_dram_parameter("w1r", [P, P], dt.bfloat16, isOutput=False)
    w2l = nc.declare_dram_parameter("w2l", [P, P], dt.bfloat16, isOutput=False)
    w2r = nc.declare_dram_parameter("w2r", [P, P], dt.bfloat16, isOutput=False)
    b1r = nc.declare_dram_parameter("b1r", [1, P], dt.bfloat16, isOutput=False)
    b2r = nc.declare_dram_parameter("b2r", [1, P], dt.bfloat16, isOutput=False)
    out = nc.declare_dram_parameter("out", [npad, D], dt.float32, isOutput=True)

    cc_in = nc.dram_tensor("cc_in", [npad, D], dt.bfloat16)
    h_full = nc.dram_tensor("h_full", [ncores * npad, D], dt.bfloat16, addr_space="Shared")

    with TileContext(nc) as tc:
        with (
            tc.tile_pool(name="persist", bufs=1) as pers,
            tc.tile_pool(name="work", bufs=6) as wp,
            tc.tile_pool(name="oh", bufs=8) as ohp,
            tc.tile_pool(name="gath1", bufs=3) as gp1,
            tc.tile_pool(name="gath2", bufs=10) as gp2,
            tc.tile_pool(name="psagg", bufs=4, space="PSUM") as psa,
            tc.tile_pool(name="pstr", bufs=2, space="PSUM") as pst,
            tc.tile_pool(name="psout", bufs=2, space="PSUM") as pso,
        ):
            def load(dram, shape, dtype, tag, eng=None):
                t = pers.tile(shape, dtype, tag=tag)
                (eng or nc.sync).dma_start(out=t[:], in_=dram[:])
                return t

            # SP queue: only what window 0's aggregation needs, then gt1
            # streaming; everything else on the scalar HWDGE queue ordered
            # by first use so layer 1 starts immediately.
            dstl1_sb = load(dstl1, [P, S1], dt.bfloat16, "dstl1")
            iota32_sb = load(iota32, [P, 32 * P], dt.bfloat16, "iota32")
            invcb_sb = load(invcb, [P, npad], dt.float32, "invcb", nc.scalar)
            xT_sb = load(xT, [P, npad], dt.bfloat16, "xT", nc.scalar)
            ones_sb = load(onesr, [1, P], dt.bfloat16, "ones", nc.scalar)
            ident_sb = load(ident, [P, P], dt.bfloat16, "ident", nc.scalar)
            idx2_sb = load(idx2, [P, l2["C"]], dt.int16, "idx2", nc.scalar)
            dstl2_sb = load(dstl2, [P, l2["TC"]], dt.bfloat16, "dstl2", nc.scalar)
            w_sb = [
                (load(w1l, [P, P], dt.bfloat16, "w1l"), load(w1r, [P, P], dt.bfloat16, "w1r"),
                 load(b1r, [1, P], dt.bfloat16, "b1r")),
                (load(w2l, [P, P], dt.bfloat16, "w2l"), load(w2r, [P, P], dt.bfloat16, "w2r"),
                 load(b2r, [1, P], dt.bfloat16, "b2r")),
            ]
            hT_sb = pers.tile([P, npad], dt.bfloat16, tag="hT")
            spill_sb = pers.tile([P, blocks * P], dt.bfloat16, tag="spill")

            # --------------- merged L1/L2 emission ----------------------
            cum_blocks_l = [r // P for r in cum_ranks]
            l1w = m["l1_windows"]
            wpasses = l2["wpasses"]
            seen_pass = [[] for _ in range(blocks)]
            qrr = [0]  # round-robin SWDGE queue

            def emit_l1_window(wi):
                wl_sb, wr_sb, br_sb = w_sb[0]
                b = wi["w"]
                s0, s1e = wi["slabs"]
                ns = s1e - s0
                blk = slice(b * P, (b + 1) * P)
                gt = gp1.tile([P, ns, D], dt.bfloat16, tag="g1")
                nc.sync.dma_start(out=gt[:, :, :],
                                  in_=gt1[:, s0 * D:s1e * D])
                ps = psa.tile([P, P], dt.float32)
                for j0 in range(0, ns, 32):
                    k = min(32, ns - j0)
                    oh = ohp.tile([P, k, P], dt.bfloat16, tag="oh")
                    nc.vector.tensor_tensor(
                        out=oh[:],
                        in0=dstl1_sb[:, s0 + j0:s0 + j0 + k]
                            .unsqueeze(2).to_broadcast([P, k, P]),
                        in1=iota32_sb[:, :k * P],
                        op=ALU.is_equal,
                    )
                    for j in range(j0, j0 + k):
                        nc.tensor.matmul(
                            out=ps[:], lhsT=gt[:, j, :], rhs=oh[:, j - j0, :],
                            start=(j == 0), stop=(j == ns - 1),
                        )
                meanT = wp.tile([P, P], dt.bfloat16, tag="meanT")
                nc.vector.tensor_tensor(
                    out=meanT[:], in0=ps[:], in1=invcb_sb[:, blk], op=ALU.mult)
                po = pso.tile([P, P], dt.float32)
                nc.tensor.matmul(out=po[:], lhsT=meanT[:], rhs=wl_sb[:], start=True, stop=False)
                nc.tensor.matmul(out=po[:], lhsT=xT_sb[:, blk], rhs=wr_sb[:], start=False, stop=False)
                nc.tensor.matmul(out=po[:], lhsT=ones_sb[:], rhs=br_sb[:], start=False, stop=True)
                mk = wp.tile([P, P], dt.bfloat16, tag="mk")
                nc.sync.dma_start(out=mk[:], in_=m2r[blk, :])
                hr = wp.tile([P, P], dt.bfloat16, tag="hr")
                nc.vector.scalar_tensor_tensor(
                    out=hr[:], in0=po[:], scalar=0.0, in1=mk[:],
                    op0=ALU.max, op1=ALU.mult)
                nc.sync.dma_start(out=cc_in[blk, :], in_=hr[:])
                tp2 = pst.tile([P, P], dt.bfloat16, tag="tp")
                nc.tensor.transpose(out=tp2[:], in_=hr[:], identity=ident_sb[:])
                nc.scalar.copy(hT_sb[:, blk], tp2[:])

            def emit_ag(q):
                nc.gpsimd.collective_compute(
                    "AllGather",
                    mybir.AluOpType.bypass,
                    ins=[cc_in[cum_ranks[q]:cum_ranks[q + 1], :]],
                    outs=[h_full[row_off[q]:row_off[q + 1], :]],
                    replica_groups=[list(range(ncores))],
                )

            def emit_l2_group(p, g):
                wl_sb, wr_sb, br_sb = w_sb[1]
                nslab = g["nslab"]
                tab = h_full[base_rows[p]:row_off[p + 1], :]
                gt = gp2.tile([P, max(nslab, 1), D], dt.bfloat16, tag="g2")
                nc.gpsimd.dma_gather(
                    out_ap=gt[:, :, :],
                    in_ap=tab,
                    idxs_ap=idx2_sb[:, g["col"]: g["col"] + nslab * 8],
                    num_idxs=nslab * P,
                    num_idxs_reg=nslab * P,
                    elem_size=D,
                    transpose=False,
                    single_packet=False,
                    queue_num=qrr[0],
                )
                qrr[0] = (qrr[0] + 1) % NQUEUES
                for wi in g["windows"]:
                    b = wi["w"]
                    blk = slice(b * P, (b + 1) * P)
                    chunks = wi["chunks"]
                    ps = psa.tile([P, P], dt.float32)
                    t0g = chunks[0][1]
                    ncols = len(chunks)
                    oh_tiles = []
                    for j0 in range(0, ncols, 32):
                        k = min(32, ncols - j0)
                        oh = ohp.tile([P, k, P], dt.bfloat16, tag="oh")
                        nc.vector.tensor_tensor(
                            out=oh[:],
                            in0=dstl2_sb[:, t0g + j0:t0g + j0 + k]
                                .unsqueeze(2).to_broadcast([P, k, P]),
                            in1=iota32_sb[:, :k * P],
                            op=ALU.is_equal,
                        )
                        oh_tiles.append(oh)
                    for ci, (slab, t_g) in enumerate(chunks):
                        oh = oh_tiles[ci // 32]
                        nc.tensor.matmul(
                            out=ps[:],
                            lhsT=gt[:, slab, :],
                            rhs=oh[:, ci % 32, :],
                            start=(ci == 0),
                            stop=(ci == len(chunks) - 1),
                        )
                    first = len(seen_pass[b]) == 0
                    last = p == wpasses[b][-1]
                    seen_pass[b].append(p)
                    if not last:
                        if first:
                            nc.scalar.copy(spill_sb[:, blk], ps[:])
                        else:
                            nc.vector.tensor_tensor(
                                out=spill_sb[:, blk], in0=ps[:],
                                in1=spill_sb[:, blk], op=ALU.add)
                    else:
                        meanT = wp.tile([P, P], dt.bfloat16, tag="meanT")
                        if first:
                            nc.vector.tensor_tensor(
                                out=meanT[:], in0=ps[:],
                                in1=invcb_sb[:, blk], op=ALU.mult)
                        else:
                            tmp = wp.tile([P, P], dt.float32, tag="tmp")
                            nc.vector.tensor_tensor(
                                out=tmp[:], in0=ps[:],
                                in1=spill_sb[:, blk], op=ALU.add)
                            nc.vector.tensor_tensor(
                                out=meanT[:], in0=tmp[:],
                                in1=invcb_sb[:, blk], op=ALU.mult)
                        po = pso.tile([P, P], dt.float32)
                        nc.tensor.matmul(out=po[:], lhsT=meanT[:], rhs=wl_sb[:], start=True, stop=False)
                        nc.tensor.matmul(out=po[:], lhsT=hT_sb[:, blk], rhs=wr_sb[:], start=False, stop=False)
                        nc.tensor.matmul(out=po[:], lhsT=ones_sb[:], rhs=br_sb[:], start=False, stop=True)
                        o = wp.tile([P, P], dt.float32, tag="o")
                        nc.scalar.copy(o[:], po[:])
                        nc.sync.dma_start(out=out[blk, :], in_=o[:])

            # L1 windows feeding the first two chunk AllGathers go first;
            # the rest are drip-fed between L2 groups so layer-2 compute
            # does not queue behind all of layer 1 on the in-order engines.
            nw = 0
            lim0 = min(cum_blocks_l[1], blocks)
            while nw < lim0:
                emit_l1_window(l1w[nw]); nw += 1
            emit_ag(0)
            DRIP = 4
            for p in range(npass):
                groups = l2["groups_by_pass"][p]
                if not groups and p + 1 < npass:
                    emit_ag(p + 1)
                for gi, g in enumerate(groups):
                    if p + 1 < npass and gi == max(0, len(groups) - 2):
                        need = (cum_blocks_l[p + 2]
                                if p + 2 < len(cum_blocks_l) else blocks)
                        while nw < min(need, blocks):
                            emit_l1_window(l1w[nw]); nw += 1
                        emit_ag(p + 1)
                    emit_l2_group(p, g)
                    for _ in range(DRIP):
                        if nw < blocks:
                            emit_l1_window(l1w[nw]); nw += 1
            while nw < blocks:
                emit_l1_window(l1w[nw]); nw += 1
    nc.compile()
    return nc


# --------------------------------------------------------------------------
# Input map construction + host post-processing
# --------------------------------------------------------------------------

def make_in_maps(meta, x, mask, W1_l, b1_l, W1_r, W2_l, b2_l, W2_r):
    m = meta
    N, ncores, npad = m["N"], m["ncores"], m["npad"]
    S1 = m["S1"]
    xb = np.asarray(x, np.float32).astype(BF16)
    w1lb = np.ascontiguousarray(np.asarray(W1_l, np.float32).astype(BF16))
    w1rb = np.ascontiguousarray(np.asarray(W1_r, np.float32).astype(BF16))
    w2lb = np.ascontiguousarray(np.asarray(W2_l, np.float32).astype(BF16))
    w2rb = np.ascontiguousarray(np.asarray(W2_r, np.float32).astype(BF16))
    b1c = np.ascontiguousarray(np.asarray(b1_l, np.float32).astype(BF16).reshape(1, P))
    b2c = np.ascontiguousarray(np.asarray(b2_l, np.float32).astype(BF16).reshape(1, P))
    iota = np.broadcast_to(np.arange(P, dtype=np.float32), (P, P)).astype(BF16)
    iota32 = np.ascontiguousarray(np.tile(iota, (1, 32)))
    onesv = np.ones((1, P), BF16)
    identv = np.ascontiguousarray(np.eye(P, dtype=np.float32).astype(BF16))
    mask2 = np.asarray(mask, np.float32) * 2.0

    maps = []
    for c in range(ncores):
        ids = m["perm"][c]
        valid = ids >= 0
        safe = np.where(valid, ids, 0)
        xp = xb[safe]
        xp[~valid] = 0
        mp = mask2[safe].astype(BF16)
        mp[~valid] = 0
        inv = m["invcnt"][safe].copy()
        inv[~valid] = 1.0

        ss = m["src_slots"][c]
        sv = ss >= 0
        ssafe = np.where(sv, ss, 0)
        rows = xb[ssafe]
        rows[~sv] = 0
        gt1 = np.ascontiguousarray(
            rows.reshape(S1, P, D).transpose(1, 0, 2).reshape(P, S1 * D))

        maps.append({
            "gt1": gt1,
            "dstl1": m["dstl1"][c],
            "idx2": m["idx2"][c], "dstl2": m["dstl2"][c],
            "xT": np.ascontiguousarray(xp.T),
            "m2r": np.ascontiguousarray(mp),
            "invcb": np.ascontiguousarray(
                np.broadcast_to(inv.astype(np.float32), (P, npad))),
            "iota32": iota32, "onesr": onesv, "ident": identv,
            "w1l": w1lb, "w1r": w1rb, "w2l": w2lb, "w2r": w2rb,
            "b1r": b1c, "b2r": b2c,
        })
    return maps


def assemble_output(meta, results):
    m = meta
    out = np.empty((m["N"], D), np.float32)
    for c in range(m["ncores"]):
        o = np.asarray(results[c]["out"], np.float32)
        ids = m["perm"][c]
        valid = ids >= 0
        out[ids[valid]] = o[valid]
    return out


# --------------------------------------------------------------------------
# Entry point
# --------------------------------------------------------------------------

def _ensure_ntff_hook():
    """Reconstruct the axon NTFF profile hook if the image lacks
    antenv.axon_hooks (degraded boot). Needed only for trace=True."""
    import types
    try:
        from antenv.axon_hooks import get_axon_ntff_profile_hook
        if get_axon_ntff_profile_hook() is not None:
            return
    except ImportError:
        mod = types.ModuleType("antenv.axon_hooks")
        holder = [None]
        mod.set_axon_ntff_profile_hook = lambda h: holder.__setitem__(0, h)
        mod.get_axon_ntff_profile_hook = lambda: holder[0]
        sys.modules["antenv.axon_hooks"] = mod
        import antenv
        antenv.axon_hooks = mod
    if "/root/.axon_site" not in sys.path:
        sys.path.insert(0, "/root/.axon_site")
    from trn_agent_boot.trn_boot import _ntff_profile_via_ctypes
    from antenv.axon_hooks import set_axon_ntff_profile_hook
    hook = _ntff_profile_via_ctypes("/opt/axon/libaxon_pjrt.so")
    set_axon_ntff_profile_hook(hook)


_CACHE = {}


def _get_ctx(edge_index, N, ncores=8):
    ei = np.asarray(edge_index, np.int64)
    key = (N, ncores, hashlib.sha1(ei.tobytes()).hexdigest())
    ctx = _CACHE.get(key)
    if ctx is None:
        meta = build_meta(ei[0], ei[1], N, ncores)
        nc = build_nc(meta)
        _CACHE.clear()
        _CACHE[key] = ctx = (meta, nc)
    return ctx


def kernel(x, edge_index, drop_mask, W1_l, b1_l, W1_r, W2_l, b2_l, W2_r,
           trace=False):
    x = np.asarray(x, np.float32)
    meta, nc = _get_ctx(edge_index, x.shape[0])
    in_maps = make_in_maps(meta, x, drop_mask, W1_l, b1_l, W1_r, W2_l, b2_l, W2_r)
    if trace:
        _ensure_ntff_hook()
    from concourse.bass_utils import run_bass_kernel_spmd
    res = run_bass_kernel_spmd(
        nc, in_maps, core_ids=list(range(meta["ncores"])), trace=trace,
    )
    out = assemble_output(meta, res.results)
    if trace:
        return out, res
    return out


# revision 7
# speedup vs baseline: 1.1385x; 1.1385x over previous
"""2-layer GraphSAGE (mean aggregation) over 8 TRN2 NeuronCores — v4.

v4 vs v3:
- num_swdge_queues=4 with round-robin queue_num on the L2 dma_gather
  calls: each SWDGE queue has a dedicated pair of Q7 DSP cores, so up
  to 4 gathers generate descriptors concurrently (the v3 gather stream
  was fully serial on one core pair at ~7.7ns/idx and was 86% of the
  kernel span).
- L2 gathers single rows (256B) instead of pairs (512B): pairing only
  deduped ~2% of indices on this random graph while doubling the
  matmul count, the DVE one-hot columns, and the gather bytes. int16
  index range is handled by making indices relative to a per-pass base
  row (each pass's gather source window spans <= 32768 h_full rows).
- One-hot IS_EQ builds are batched 16 slabs per DVE instruction.
- PSUM->SBUF copies (output tile, hT transpose, spill init) moved to
  the otherwise-idle Scalar (Activation) engine.

Layer 1 is host-pregathered (the gather pattern depends only on
edge_index): the device streams pre-packed slab tables with plain HWDGE
DMA. Aggregation matmuls accumulate agg^T[feat, dst] with the gathered
slab as lhsT and the one-hot as rhs; mean scaling uses a
host-replicated [P, npad] inv-degree table.
"""

import sys

for _p in ("/opt/trn_rl_repo",):
    if _p not in sys.path:
        sys.path.insert(0, _p)

import hashlib
import numpy as np
import ml_dtypes

BF16 = ml_dtypes.bfloat16
P = 128
D = 128

SLAB_BUDGET = 20   # max 128-slot slabs per L2 gather call
NQUEUES = 4
IDX_SPAN = 32768   # int16 index reach per gather call (rows)


# --------------------------------------------------------------------------
# Host-side schedule construction
# --------------------------------------------------------------------------

def build_meta(src, dst, N, ncores):
    src = np.asarray(src, np.int64)
    dst = np.asarray(dst, np.int64)
    npc = N // ncores
    assert npc * ncores == N, (N, ncores)
    blocks = -(-npc // P)
    npad = blocks * P
    assert npc < npad, "need pad ranks for layer-2 zero sentinels"

    # uneven source chunks (in blocks): small first chunks so the first
    # AllGather — which gates the start of dma_gather descriptor
    # generation — completes as early as possible. Consecutive chunk
    # pairs stay under IDX_SPAN rows so per-pass relative int16 indices
    # can reach back across one chunk boundary.
    cuts = sorted({max(1, (blocks * num) // den) for num, den in
                   ((5, 49), (14, 49), (27, 49), (41, 49), (1, 1))})
    if cuts[-1] != blocks:
        cuts.append(blocks)
    cum_blocks = [0] + cuts
    nchunk = len(cuts)
    chunk_ranks = [cum_blocks[i + 1] * P - cum_blocks[i] * P
                   for i in range(nchunk)]
    cum_ranks = [cb * P for cb in cum_blocks]            # per-core rank cuts
    row_off = [ncores * cr for cr in cum_ranks]          # h_full row offsets
    assert npc > cum_ranks[-2], "pad ranks must fall in the last chunk"
    # per-pass gather base rows (indices are relative to these)
    base_rows = [max(0, row_off[p + 1] - IDX_SPAN) for p in range(nchunk)]
    for p in range(nchunk):
        assert row_off[p + 1] - base_rows[p] <= IDX_SPAN

    deg = np.bincount(dst, minlength=N)
    invcnt = (1.0 / np.maximum(deg, 1.0)).astype(np.float32)

    perm = -np.ones((ncores, npad), np.int64)
    for c in range(ncores):
        perm[c, :npc] = np.arange(c * npc, (c + 1) * npc)
    rank = np.mod(np.arange(N), npc)
    core_of = np.arange(N) // npc
    q_of = np.searchsorted(cum_ranks, rank, side="right") - 1
    q_of = np.minimum(q_of, nchunk - 1)
    cr = np.asarray(cum_ranks[:-1], np.int64)[q_of]
    crk = np.asarray(chunk_ranks, np.int64)[q_of]
    pos = (np.asarray(row_off[:-1], np.int64)[q_of]
           + core_of * crk + (rank - cr))  # chunk-major h_full row

    # ---------------- Layer 1: host-pregathered, unpaired slots ----------
    per_core_l1 = []
    nch1 = np.zeros(blocks, np.int64)
    for c in range(ncores):
        sel = core_of[dst] == c
        s_c = src[sel]
        r = rank[dst[sel]]
        w = r // P
        dl = r % P
        order = np.argsort(w, kind="stable")
        sw = w[order]
        cntw = np.bincount(sw, minlength=blocks)
        np.maximum(nch1, -(-cntw // P), out=nch1)
        per_core_l1.append((s_c[order], sw, dl[order]))
    nch1 = np.maximum(nch1, 1)
    slab_of1 = np.concatenate([[0], np.cumsum(nch1)[:-1]])
    S1 = int(nch1.sum())

    src_slots = []
    dstl1 = []
    for c in range(ncores):
        s_sorted, sw, dl_sorted = per_core_l1[c]
        flat_src = -np.ones(S1 * P, np.int64)
        flat_dl = np.full(S1 * P, 255, np.int64)
        wstart = np.searchsorted(sw, np.arange(blocks))
        posn = slab_of1[sw] * P + (np.arange(len(sw)) - wstart[sw])
        flat_src[posn] = s_sorted
        flat_dl[posn] = dl_sorted
        src_slots.append(flat_src)
        dstl1.append(np.ascontiguousarray(
            flat_dl.reshape(S1, P).T.astype(BF16)))

    l1_windows = [{"w": b, "slabs": (int(slab_of1[b]), int(slab_of1[b] + nch1[b]))}
                  for b in range(blocks)]

    # ---------------- Layer 2: single-row slots, passes by source chunk --
    NPASS_L2 = nchunk
    nch = np.zeros(blocks, np.int64)
    per_core = []
    for c in range(ncores):
        sel = core_of[dst] == c
        s_c = src[sel]
        r = rank[dst[sel]]
        row = pos[s_c]                      # h_full source row per edge
        w = r // P
        dl = r % P
        okey = (w << 16) | row
        order = np.argsort(okey, kind="stable")
        sk = okey[order]
        # one slot per edge, sorted by (window, row)
        uw = (sk >> 16).astype(np.int64)
        cntw = np.bincount(uw, minlength=blocks)
        np.maximum(nch, -(-cntw // P), out=nch)
        per_core.append((sk & 0xFFFF, uw, dl[order]))
    nch = np.maximum(nch, 1)

    # shared per-(window, slab) pass assignment: max over cores of the
    # chunk of the slab's last real slot for that core
    slab_pass = [np.zeros(int(nch[b]), np.int64) for b in range(blocks)]
    for c in range(ncores):
        rows, uw, dl = per_core[c]
        uq = np.searchsorted(row_off, rows, side="right") - 1
        wfirst = np.searchsorted(uw, np.arange(blocks))
        wcnt = np.bincount(uw, minlength=blocks)
        for b in range(blocks):
            n = int(wcnt[b])
            if n == 0:
                continue
            qs = uq[wfirst[b]:wfirst[b] + n]
            for j in range(int(nch[b])):
                if 128 * j >= n:
                    break
                last = min(128 * (j + 1) - 1, n - 1)
                slab_pass[b][j] = max(slab_pass[b][j], qs[last])

    # global slab order: pass-major, then window
    slab_gid = [np.zeros(int(nch[b]), np.int64) for b in range(blocks)]
    slab_pass_flat = {}
    runs_by_pass = [[] for _ in range(NPASS_L2)]  # (w, j0, j1, t0)
    t = 0
    for p in range(NPASS_L2):
        for b in range(blocks):
            js = np.nonzero(slab_pass[b] == p)[0]
            if len(js) == 0:
                continue
            j0, j1 = int(js[0]), int(js[-1]) + 1
            assert list(js) == list(range(j0, j1))
            slab_gid[b][j0:j1] = np.arange(t, t + (j1 - j0))
            for g in range(t, t + (j1 - j0)):
                slab_pass_flat[g] = p
            runs_by_pass[p].append((b, j0, j1, t))
            t += j1 - j0
    total_slabs = t
    assert total_slabs == int(nch.sum())

    # groups (gather calls): pack whole runs under SLAB_BUDGET, per pass
    groups_by_pass = []
    for p in range(NPASS_L2):
        groups = []
        cur = None
        for (b, j0, j1, t0) in runs_by_pass[p]:
            n = j1 - j0
            assert n <= SLAB_BUDGET
            if cur is None or cur["nslab"] + n > SLAB_BUDGET:
                cur = {"base": t0, "nslab": 0, "windows": []}
                groups.append(cur)
            cur["windows"].append(
                {"w": b, "chunks": [(t0 - cur["base"] + j, t0 + j)
                                    for j in range(n)]})
            cur["nslab"] += n
        for g in groups:
            g["col"] = g["base"] * 8
            g["NI"] = g["nslab"] * P
        groups_by_pass.append(groups)

    # per-window pass bookkeeping
    wpasses = [[] for _ in range(blocks)]
    for p in range(NPASS_L2):
        for (b, j0, j1, t0) in runs_by_pass[p]:
            wpasses[b].append(p)

    # per-core idx / dstl tables in the global slab order; indices are
    # relative to base_rows[pass(slab)]
    idx2 = []
    dstl2 = []
    for c in range(ncores):
        rows, uw, dl = per_core[c]
        flat = np.zeros(total_slabs * P, np.int64)
        dst2 = np.full(total_slabs * P, 255, np.int64)
        wfirst = np.searchsorted(uw, np.arange(blocks))
        iw = np.arange(len(uw)) - wfirst[uw]   # position within window
        jloc = iw // P
        within = iw % P
        gsl = np.empty(len(uw), np.int64)
        for b in range(blocks):
            m = uw == b
            if m.any():
                gsl[m] = slab_gid[b][jloc[m]]
        spos = gsl * P + within
        gbase = np.asarray([base_rows[slab_pass_flat[g]]
                            for g in range(total_slabs)], np.int64)
        rel = rows - gbase[gsl]
        assert (rel >= 0).all() and (rel < IDX_SPAN).all(), \
            "slot row outside its pass's index window"
        flat[spos] = rel
        dst2[spos] = dl
        idx2.append(np.ascontiguousarray(
            np.tile(flat.reshape(-1, 16).T.astype(np.int16), (8, 1))))
        dstl2.append(np.ascontiguousarray(
            dst2.reshape(total_slabs, P).T.astype(BF16)))

    layer2 = {"groups_by_pass": groups_by_pass, "C": total_slabs * 8,
              "TC": total_slabs, "wpasses": wpasses}

    return {
        "N": N, "ncores": ncores, "blocks": blocks, "npad": npad,
        "cum_ranks": cum_ranks, "row_off": row_off, "npass": NPASS_L2,
        "base_rows": base_rows,
        "perm": perm, "invcnt": invcnt,
        "S1": S1, "l1_windows": l1_windows,
        "src_slots": src_slots, "dstl1": dstl1,
        "layer2": layer2, "idx2": idx2, "dstl2": dstl2,
    }


# --------------------------------------------------------------------------
# Bass graph
# --------------------------------------------------------------------------

def build_nc(meta):
    from concourse import bacc, mybir
    from concourse.tile import TileContext

    dt = mybir.dt
    ALU = mybir.AluOpType
    ACT = mybir.ActivationFunctionType
    m = meta
    npad, ncores, blocks = m["npad"], m["ncores"], m["blocks"]
    cum_ranks, row_off = m["cum_ranks"], m["row_off"]
    base_rows = m["base_rows"]
    npass = m["npass"]
    S1 = m["S1"]
    l2 = m["layer2"]

    nc = bacc.Bacc(num_swdge_queues=NQUEUES)

    gt1 = nc.declare_dram_parameter("gt1", [P, S1 * D], dt.bfloat16, isOutput=False)
    dstl1 = nc.declare_dram_parameter("dstl1", [P, S1], dt.bfloat16, isOutput=False)
    idx2 = nc.declare_dram_parameter("idx2", [P, l2["C"]], dt.int16, isOutput=False)
    dstl2 = nc.declare_dram_parameter("dstl2", [P, l2["TC"]], dt.bfloat16, isOutput=False)
    xT = nc.declare_dram_parameter("xT", [P, npad], dt.bfloat16, isOutput=False)
    m2r = nc.declare_dram_parameter("m2r", [npad, D], dt.bfloat16, isOutput=False)
    invcb = nc.declare_dram_parameter("invcb", [P, npad], dt.bfloat16, isOutput=False)
    iota32 = nc.declare_dram_parameter("iota32", [P, 32 * P], dt.bfloat16, isOutput=False)
    onesr = nc.declare_dram_parameter("onesr", [1, P], dt.bfloat16, isOutput=False)
    ident = nc.declare_dram_parameter("ident", [P, P], dt.bfloat16, isOutput=False)
    w1l = nc.declare_dram_parameter("w1l", [P, P], dt.bfloat16, isOutput=False)
    w1r = nc.declare_dram_parameter("w1r", [P, P], dt.bfloat16, isOutput=False)
    w2l = nc.declare_dram_parameter("w2l", [P, P], dt.bfloat16, isOutput=False)
    w2r = nc.declare_dram_parameter("w2r", [P, P], dt.bfloat16, isOutput=False)
    b1r = nc.declare_dram_parameter("b1r", [1, P], dt.bfloat16, isOutput=False)
    b2r = nc.declare_dram_parameter("b2r", [1, P], dt.bfloat16, isOutput=False)
    out = nc.declare_dram_parameter("out", [npad, D], dt.float32, isOutput=True)

    cc_in = nc.dram_tensor("cc_in", [npad, D], dt.bfloat16)
    h_full = nc.dram_tensor("h_full", [ncores * npad, D], dt.bfloat16, addr_space="Shared")

    with TileContext(nc) as tc:
        with (
            tc.tile_pool(name="persist", bufs=1) as pers,
            tc.tile_pool(name="work", bufs=6) as wp,
            tc.tile_pool(name="oh", bufs=8) as ohp,
            tc.tile_pool(name="gath1", bufs=3) as gp1,
            tc.tile_pool(name="gath2", bufs=12) as gp2,
            tc.tile_pool(name="psagg", bufs=4, space="PSUM") as psa,
            tc.tile_pool(name="pstr", bufs=2, space="PSUM") as pst,
            tc.tile_pool(name="psout", bufs=2, space="PSUM") as pso,
        ):
            def load(dram, shape, dtype, tag, eng=None):
                t = pers.tile(shape, dtype, tag=tag)
                (eng or nc.sync).dma_start(out=t[:], in_=dram[:])
                return t

            # SP queue: only what window 0's aggregation needs, then gt1
            # streaming; everything else on the scalar HWDGE queue ordered
            # by first use so layer 1 starts immediately.
            dstl1_sb = load(dstl1, [P, S1], dt.bfloat16, "dstl1")
            iota32_sb = load(iota32, [P, 32 * P], dt.bfloat16, "iota32")
            invcb_sb = load(invcb, [P, npad], dt.bfloat16, "invcb", nc.scalar)
            xT_sb = load(xT, [P, npad], dt.bfloat16, "xT", nc.scalar)
            ones_sb = load(onesr, [1, P], dt.bfloat16, "ones", nc.scalar)
            ident_sb = load(ident, [P, P], dt.bfloat16, "ident", nc.scalar)
            idx2_sb = load(idx2, [P, l2["C"]], dt.int16, "idx2", nc.scalar)
            dstl2_sb = load(dstl2, [P, l2["TC"]], dt.bfloat16, "dstl2", nc.scalar)
            w_sb = [
                (load(w1l, [P, P], dt.bfloat16, "w1l"), load(w1r, [P, P], dt.bfloat16, "w1r"),
                 load(b1r, [1, P], dt.bfloat16, "b1r")),
                (load(w2l, [P, P], dt.bfloat16, "w2l"), load(w2r, [P, P], dt.bfloat16, "w2r"),
                 load(b2r, [1, P], dt.bfloat16, "b2r")),
            ]
            hT_sb = pers.tile([P, npad], dt.bfloat16, tag="hT")
            spill_sb = pers.tile([P, blocks * P], dt.bfloat16, tag="spill")

            # --------------- merged L1/L2 emission ----------------------
            cum_blocks_l = [r // P for r in cum_ranks]
            l1w = m["l1_windows"]
            wpasses = l2["wpasses"]
            seen_pass = [[] for _ in range(blocks)]
            qrr = [0]  # round-robin SWDGE queue

            def emit_l1_window(wi):
                wl_sb, wr_sb, br_sb = w_sb[0]
                b = wi["w"]
                s0, s1e = wi["slabs"]
                ns = s1e - s0
                blk = slice(b * P, (b + 1) * P)
                gt = gp1.tile([P, ns, D], dt.bfloat16, tag="g1")
                nc.sync.dma_start(out=gt[:, :, :],
                                  in_=gt1[:, s0 * D:s1e * D])
                ps = psa.tile([P, P], dt.float32)
                for j0 in range(0, ns, 32):
                    k = min(32, ns - j0)
                    oh = ohp.tile([P, k, P], dt.bfloat16, tag="oh")
                    nc.vector.tensor_tensor(
                        out=oh[:],
                        in0=dstl1_sb[:, s0 + j0:s0 + j0 + k]
                            .unsqueeze(2).to_broadcast([P, k, P]),
                        in1=iota32_sb[:, :k * P],
                        op=ALU.is_equal,
                    )
                    for j in range(j0, j0 + k):
                        nc.tensor.matmul(
                            out=ps[:], lhsT=gt[:, j, :], rhs=oh[:, j - j0, :],
                            start=(j == 0), stop=(j == ns - 1),
                        )
                meanT = wp.tile([P, P], dt.bfloat16, tag="meanT")
                nc.vector.tensor_tensor(
                    out=meanT[:], in0=ps[:], in1=invcb_sb[:, blk], op=ALU.mult)
                po = pso.tile([P, P], dt.float32)
                nc.tensor.matmul(out=po[:], lhsT=meanT[:], rhs=wl_sb[:], start=True, stop=False)
                nc.tensor.matmul(out=po[:], lhsT=xT_sb[:, blk], rhs=wr_sb[:], start=False, stop=False)
                nc.tensor.matmul(out=po[:], lhsT=ones_sb[:], rhs=br_sb[:], start=False, stop=True)
                mk = wp.tile([P, P], dt.bfloat16, tag="mk")
                nc.sync.dma_start(out=mk[:], in_=m2r[blk, :])
                hr = wp.tile([P, P], dt.bfloat16, tag="hr")
                nc.vector.scalar_tensor_tensor(
                    out=hr[:], in0=po[:], scalar=0.0, in1=mk[:],
                    op0=ALU.max, op1=ALU.mult)
                nc.sync.dma_start(out=cc_in[blk, :], in_=hr[:])
                tp2 = pst.tile([P, P], dt.bfloat16, tag="tp")
                nc.tensor.transpose(out=tp2[:], in_=hr[:], identity=ident_sb[:])
                nc.scalar.copy(hT_sb[:, blk], tp2[:])

            def emit_ag(q):
                nc.gpsimd.collective_compute(
                    "AllGather",
                    mybir.AluOpType.bypass,
                    ins=[cc_in[cum_ranks[q]:cum_ranks[q + 1], :]],
                    outs=[h_full[row_off[q]:row_off[q + 1], :]],
                    replica_groups=[list(range(ncores))],
                )

            def emit_l2_group(p, g):
                wl_sb, wr_sb, br_sb = w_sb[1]
                nslab = g["nslab"]
                tab = h_full[base_rows[p]:row_off[p + 1], :]
                gt = gp2.tile([P, max(nslab, 1), D], dt.bfloat16, tag="g2")
                nc.gpsimd.dma_gather(
                    out_ap=gt[:, :, :],
                    in_ap=tab,
                    idxs_ap=idx2_sb[:, g["col"]: g["col"] + nslab * 8],
                    num_idxs=nslab * P,
                    num_idxs_reg=nslab * P,
                    elem_size=D,
                    transpose=False,
                    single_packet=False,
                    queue_num=qrr[0],
                )
                qrr[0] = (qrr[0] + 1) % NQUEUES
                for wi in g["windows"]:
                    b = wi["w"]
                    blk = slice(b * P, (b + 1) * P)
                    chunks = wi["chunks"]
                    ps = psa.tile([P, P], dt.float32)
                    t0g = chunks[0][1]
                    ncols = len(chunks)
                    oh_tiles = []
                    for j0 in range(0, ncols, 32):
                        k = min(32, ncols - j0)
                        oh = ohp.tile([P, k, P], dt.bfloat16, tag="oh")
                        nc.vector.tensor_tensor(
                            out=oh[:],
                            in0=dstl2_sb[:, t0g + j0:t0g + j0 + k]
                                .unsqueeze(2).to_broadcast([P, k, P]),
                            in1=iota32_sb[:, :k * P],
                            op=ALU.is_equal,
                        )
                        oh_tiles.append(oh)
                    for ci, (slab, t_g) in enumerate(chunks):
                        oh = oh_tiles[ci // 32]
                        nc.tensor.matmul(
                            out=ps[:],
                            lhsT=gt[:, slab, :],
                            rhs=oh[:, ci % 32, :],
                            start=(ci == 0),
                            stop=(ci == len(chunks) - 1),
                        )
                    first = len(seen_pass[b]) == 0
                    last = p == wpasses[b][-1]
                    seen_pass[b].append(p)
                    if not last:
                        if first:
                            nc.scalar.copy(spill_sb[:, blk], ps[:])
                        else:
                            nc.vector.tensor_tensor(
                                out=spill_sb[:, blk], in0=ps[:],
                                in1=spill_sb[:, blk], op=ALU.add)
                    else:
                        meanT = wp.tile([P, P], dt.bfloat16, tag="meanT")
                        if first:
                            nc.vector.tensor_tensor(
                                out=meanT[:], in0=ps[:],
                                in1=invcb_sb[:, blk], op=ALU.mult)
                        else:
                            tmp = wp.tile([P, P], dt.float32, tag="tmp")
                            nc.vector.tensor_tensor(
                                out=tmp[:], in0=ps[:],
                                in1=spill_sb[:, blk], op=ALU.add)
                            nc.vector.tensor_tensor(
                                out=meanT[:], in0=tmp[:],
                                in1=invcb_sb[:, blk], op=ALU.mult)
                        po = pso.tile([P, P], dt.float32)
                        nc.tensor.matmul(out=po[:], lhsT=meanT[:], rhs=wl_sb[:], start=True, stop=False)
                        nc.tensor.matmul(out=po[:], lhsT=hT_sb[:, blk], rhs=wr_sb[:], start=False, stop=False)
                        nc.tensor.matmul(out=po[:], lhsT=ones_sb[:], rhs=br_sb[:], start=False, stop=True)
                        o = wp.tile([P, P], dt.float32, tag="o")
                        nc.scalar.copy(o[:], po[:])
                        nc.sync.dma_start(out=out[blk, :], in_=o[:])

            # L1 windows feeding the first two chunk AllGathers go first;
            # the rest are drip-fed between L2 groups so layer-2 compute
            # does not queue behind all of layer 1 on the in-order engines.
            nw = 0
            lim0 = min(cum_blocks_l[1], blocks)
            while nw < lim0:
                emit_l1_window(l1w[nw]); nw += 1
            emit_ag(0)
            front = min(cum_blocks_l[3] if len(cum_blocks_l) > 3 else blocks,
                        blocks)
            while nw < front:
                emit_l1_window(l1w[nw]); nw += 1
            DRIP = 2
            for p in range(npass):
                groups = l2["groups_by_pass"][p]
                if not groups and p + 1 < npass:
                    emit_ag(p + 1)
                for gi, g in enumerate(groups):
                    if p + 1 < npass and gi == max(0, len(groups) - 2):
                        need = (cum_blocks_l[p + 2]
                                if p + 2 < len(cum_blocks_l) else blocks)
                        while nw < min(need, blocks):
                            emit_l1_window(l1w[nw]); nw += 1
                        emit_ag(p + 1)
                    emit_l2_group(p, g)
                    for _ in range(DRIP):
                        if nw < blocks:
                            emit_l1_window(l1w[nw]); nw += 1
            while nw < blocks:
                emit_l1_window(l1w[nw]); nw += 1
    nc.compile()
    return nc


# --------------------------------------------------------------------------
# Input map construction + host post-processing
# --------------------------------------------------------------------------

def make_in_maps(meta, x, mask, W1_l, b1_l, W1_r, W2_l, b2_l, W2_r):
    m = meta
    N, ncores, npad = m["N"], m["ncores"], m["npad"]
    S1 = m["S1"]
    xb = np.asarray(x, np.float32).astype(BF16)
    w1lb = np.ascontiguousarray(np.asarray(W1_l, np.float32).astype(BF16))
    w1rb = np.ascontiguousarray(np.asarray(W1_r, np.float32).astype(BF16))
    w2lb = np.ascontiguousarray(np.asarray(W2_l, np.float32).astype(BF16))
    w2rb = np.ascontiguousarray(np.asarray(W2_r, np.float32).astype(BF16))
    b1c = np.ascontiguousarray(np.asarray(b1_l, np.float32).astype(BF16).reshape(1, P))
    b2c = np.ascontiguousarray(np.asarray(b2_l, np.float32).astype(BF16).reshape(1, P))
    iota = np.broadcast_to(np.arange(P, dtype=np.float32), (P, P)).astype(BF16)
    iota32 = np.ascontiguousarray(np.tile(iota, (1, 32)))
    onesv = np.ones((1, P), BF16)
    identv = np.ascontiguousarray(np.eye(P, dtype=np.float32).astype(BF16))
    mask2 = np.asarray(mask, np.float32) * 2.0

    maps = []
    for c in range(ncores):
        ids = m["perm"][c]
        valid = ids >= 0
        safe = np.where(valid, ids, 0)
        xp = xb[safe]
        xp[~valid] = 0
        mp = mask2[safe].astype(BF16)
        mp[~valid] = 0
        inv = m["invcnt"][safe].copy()
        inv[~valid] = 1.0

        ss = m["src_slots"][c]
        sv = ss >= 0
        ssafe = np.where(sv, ss, 0)
        rows = xb[ssafe]
        rows[~sv] = 0
        gt1 = np.ascontiguousarray(
            rows.reshape(S1, P, D).transpose(1, 0, 2).reshape(P, S1 * D))

        maps.append({
            "gt1": gt1,
            "dstl1": m["dstl1"][c],
            "idx2": m["idx2"][c], "dstl2": m["dstl2"][c],
            "xT": np.ascontiguousarray(xp.T),
            "m2r": np.ascontiguousarray(mp),
            "invcb": np.ascontiguousarray(
                np.broadcast_to(inv.astype(BF16), (P, npad))),
            "iota32": iota32, "onesr": onesv, "ident": identv,
            "w1l": w1lb, "w1r": w1rb, "w2l": w2lb, "w2r": w2rb,
            "b1r": b1c, "b2r": b2c,
        })
    return maps


def assemble_output(meta, results):
    m = meta
    out = np.empty((m["N"], D), np.float32)
    for c in range(m["ncores"]):
        o = np.asarray(results[c]["out"], np.float32)
        ids = m["perm"][c]
        valid = ids >= 0
        out[ids[valid]] = o[valid]
    return out


# --------------------------------------------------------------------------
# Entry point
# --------------------------------------------------------------------------

def _ensure_ntff_hook():
    """Reconstruct the axon NTFF profile hook if the image lacks
    antenv.axon_hooks (degraded boot). Needed only for trace=True."""
    import types
    try:
        from antenv.axon_hooks import get_axon_ntff_profile_hook
        if get_axon_ntff_profile_hook() is not None:
            return
    except ImportError:
        mod = types.ModuleType("antenv.axon_hooks")
        holder = [None]
        mod.set_axon_ntff_profile_hook = lambda h: holder.__setitem__(0, h)
        mod.get_axon_ntff_profile_hook = lambda: holder[0]
        sys.modules["antenv.axon_hooks"] = mod
        import antenv
        antenv.axon_hooks = mod
    if "/root/.axon_site" not in sys.path:
        sys.path.insert(0, "/root/.axon_site")
    from trn_agent_boot.trn_boot import _ntff_profile_via_ctypes
    from antenv.axon_hooks import set_axon_ntff_profile_hook
    hook = _ntff_profile_via_ctypes("/opt/axon/libaxon_pjrt.so")
    set_axon_ntff_profile_hook(hook)


_CACHE = {}


def _get_ctx(edge_index, N, ncores=8):
    ei = np.asarray(edge_index, np.int64)
    key = (N, ncores, hashlib.sha1(ei.tobytes()).hexdigest())
    ctx = _CACHE.get(key)
    if ctx is None:
        meta = build_meta(ei[0], ei[1], N, ncores)
        nc = build_nc(meta)
        _CACHE.clear()
        _CACHE[key] = ctx = (meta, nc)
    return ctx


def kernel(x, edge_index, drop_mask, W1_l, b1_l, W1_r, W2_l, b2_l, W2_r,
           trace=False):
    x = np.asarray(x, np.float32)
    meta, nc = _get_ctx(edge_index, x.shape[0])
    in_maps = make_in_maps(meta, x, drop_mask, W1_l, b1_l, W1_r, W2_l, b2_l, W2_r)
    if trace:
        _ensure_ntff_hook()
    from concourse.bass_utils import run_bass_kernel_spmd
    res = run_bass_kernel_spmd(
        nc, in_maps, core_ids=list(range(meta["ncores"])), trace=trace,
    )
    out = assemble_output(meta, res.results)
    if trace:
        return out, res
    return out
